# revision 14
# baseline (speedup 1.0000x reference)
"""CrystalEncoder Trainium2 kernel: 8-way data-parallel over graphs.

Per core: 8 graphs = 2048 nodes, 32768 edges per edge set.
  - node feature tables in HBM (fp16, 128-col rows); edge gathers via
    dma_gather(transpose=True) -> [feat(part), edge(free)] fp16 tiles
  - MLP layers as fp16 matmuls (fp32 psum), feature-on-partition
  - BatchNorm (training-mode batch stats) folded into the next layer's
    weights; per-layer stats: ACT relu+accum (sum) + DVE STT (sumsq);
    cross-core AllGather of tiny [128, 2*nch] partials
  - segment-max aggregation commuted past relu/BN (monotone) -> window
    max trees on DVE
  - pooled [B,1024] features AllGather'ed, head replicated on all cores
"""

import numpy as np
from contextlib import ExitStack

import concourse.bass as bass
import concourse.bacc as bacc
import concourse.tile as tile
from concourse import mybir
from concourse.bass_utils import run_bass_kernel_spmd

F16 = mybir.dt.float16
F32 = mybir.dt.float32
I16 = mybir.dt.int16
AF = mybir.ActivationFunctionType
ALU = mybir.AluOpType
AX = mybir.AxisListType

NCORE = 8
B = 64
PTS = 256
NTOTN = B * PTS       # 16384
DEG = 16
FA = 98
NG = B // NCORE       # 8 graphs per core
NN = NG * PTS         # 2048 nodes per core
NE = NN * DEG         # 32768 edges per core
EPS = 1e-5
NBAT_E = float(NTOTN * DEG)
NBAT_N = float(NTOTN)

BLK = 1024            # psum block columns (2 banks)
GCH = 512             # gather chunk (idxs; SWDGE desc carveout is 1024)


def _wrap_idx(ix):
    """dma_gather index layout: idx i at [i%16, i//16], tiled to 128 parts."""
    t = ix.astype(np.int16).reshape(-1, 16).T
    return np.tile(t, (8, 1)).copy()


class _Blob:
    def __init__(self, dtype):
        self.dtype = dtype
        self.cols = []
        self.slices = {}
        self.c = 0

    def add(self, name, arr):
        a = np.asarray(arr, dtype=self.dtype)
        assert a.ndim == 2 and a.shape[0] <= 128, (name, a.shape)
        p = np.zeros((128, a.shape[1]), dtype=self.dtype)
        p[: a.shape[0], :] = a
        self.cols.append(p)
        self.slices[name] = (self.c, a.shape[1], a.shape[0])
        self.c += a.shape[1]

    def build(self):
        return np.concatenate(self.cols, axis=1)


def _host_prep(inputs):
    x = np.asarray(inputs["x"], np.float32)
    pos = np.asarray(inputs["pos"], np.float32)
    e1src = np.asarray(inputs["e1_src"], np.int32)
    e2src = np.asarray(inputs["e2_src"], np.int32)

    def WBGE(layer):
        return tuple(np.asarray(a, np.float32) for a in layer)

    m1 = [WBGE(l) for l in inputs["mlp1"]]
    m2 = [WBGE(l) for l in inputs["mlp2"]]
    m3 = [WBGE(l) for l in inputs["mlp3"]]
    L1W = np.asarray(inputs["lin1"][0], np.float32)
    L2W = np.asarray(inputs["lin2"][0], np.float32)
    RW = np.asarray(inputs["reg"][0], np.float32)
    Rb = np.asarray(inputs["reg"][1], np.float32)
    gh1, bh1 = (np.asarray(a, np.float32) for a in inputs["bn1"])
    gh2, bh2 = (np.asarray(a, np.float32) for a in inputs["bn2"])

    dup = lambda v: np.concatenate([v, v])
    col = lambda v: v.reshape(-1, 1)
    row = lambda v: v.reshape(1, -1)
    chcol = lambda v, nch: v.reshape(nch, 128).T

    ba = _Blob(np.float32)          # conv-phase f32 blob
    ps = np.zeros((128, 128), np.float32)
    for k in range(128):
        ps[k, k % 64] = 1.0
        ps[k, k % 64 + 64] = 1.0
    ba.add("pairsum", ps)
    ba.add("w12d", np.tile(m1[1][0], (2, 1)))
    ba.add("w12s", np.concatenate([m1[1][0], m1[1][0]], 1))
    ba.add("w13d", np.tile(m1[2][0], (2, 1)))
    ba.add("w2x", m2[0][0][:128])
    ba.add("w22", m2[1][0])
    ba.add("w23", m2[2][0])
    ba.add("b12r", row(dup(m1[1][1])))
    ba.add("b13r", row(m1[2][1]))
    ba.add("b21r", row(m2[0][1]))
    ba.add("b22r", row(m2[1][1]))
    ba.add("b23r", row(m2[2][1]))
    ba.add("g11", col(dup(m1[0][2]))); ba.add("be11", col(dup(m1[0][3])))
    ba.add("g12", col(dup(m1[1][2]))); ba.add("be12", col(dup(m1[1][3])))
    ba.add("g13", col(m1[2][2])); ba.add("be13", col(m1[2][3]))
    ba.add("g21", col(m2[0][2])); ba.add("be21", col(m2[0][3]))
    ba.add("g22", col(m2[1][2])); ba.add("be22", col(m2[1][3]))
    ba.add("g23", chcol(m2[2][2], 2)); ba.add("be23", chcol(m2[2][3], 2))
    blob32a = ba.build()

    bb = _Blob(np.float32)          # mlp3/head-phase f32 blob
    bb.add("w3a", np.concatenate([m3[0][0][0:128], m3[0][0][128:256]], 1))
    bb.add("w3b", np.concatenate([m3[1][0][0:128], m3[1][0][128:256]], 1))
    bb.add("w3c", np.concatenate([m3[2][0][k * 128:(k + 1) * 128] for k in range(4)], 1))
    bb.add("b31r", row(m3[0][1]))
    bb.add("b32r", row(m3[1][1]))
    bb.add("b33r", row(m3[2][1]))
    bb.add("g31", chcol(m3[0][2], 2)); bb.add("be31", chcol(m3[0][3], 2))
    bb.add("g32", chcol(m3[1][2], 4)); bb.add("be32", chcol(m3[1][3], 4))
    bb.add("g33", chcol(m3[2][2], 8)); bb.add("be33", chcol(m3[2][3], 8))
    bb.add("gh1", chcol(gh1, 4)); bb.add("bh1", chcol(bh1, 4))
    bb.add("gh2", chcol(gh2, 4)); bb.add("bh2", chcol(bh2, 4))
    bb.add("regb", np.array([[float(Rb[0])]], np.float32))
    blob32b = bb.build()

    bc = _Blob(np.float16)          # conv-phase fp16 blob
    w1ext = np.zeros((128, 64), np.float32)
    w1ext[0:101] = m1[0][0]
    w1ext[101] = m1[0][1]
    bc.add("w1ext", w1ext)
    w1dn = np.zeros((128, 64), np.float32)
    w1dn[98:101] = -m1[0][0][98:101]
    bc.add("w1dn", w1dn)
    w2p4 = np.zeros((4, 128), np.float32)
    w2p4[0:3] = m2[0][0][128:131]
    bc.add("w2p4", w2p4)
    w3p4 = np.zeros((4, 256), np.float32)
    w3p4[0:3] = m3[0][0][256:259]
    bc.add("w3p4", w3p4)
    bc.add("id4", np.eye(4, dtype=np.float32))
    blob16a = bc.build()

    bd = _Blob(np.float16)          # head fp16 blob
    bd.add("lin1", np.concatenate([L1W[k * 128:(k + 1) * 128] for k in range(8)], 1))
    bd.add("lin2", np.concatenate([L2W[k * 128:(k + 1) * 128] for k in range(4)], 1))
    bd.add("reg", np.concatenate([RW[k * 128:(k + 1) * 128] for k in range(4)], 1))
    blob16b = bd.build()

    dst = _wrap_idx(np.arange(NE, dtype=np.int32) // DEG)
    in_maps = []
    for c in range(NCORE):
        ns = c * NN
        es = c * NE
        in_maps.append({
            "x": np.ascontiguousarray(x[ns:ns + NN]),
            "posT": np.ascontiguousarray(pos[ns:ns + NN].T),
            "e1s": _wrap_idx(e1src[es:es + NE] - ns),
            "e2s": _wrap_idx(e2src[es:es + NE] - ns),
            "edst": dst,
            "blob32a": blob32a,
            "blob32b": blob32b,
            "blob16a": blob16a,
            "blob16b": blob16b,
        })
    shapes = (ba.slices, bb.slices, bc.slices, bd.slices,
              blob32a.shape[1], blob32b.shape[1], blob16a.shape[1], blob16b.shape[1])
    return in_maps, shapes


def build_kernel(nc, shapes):
    SA, SB, SC, SD, CA, CB, CC, CD = shapes
    xin = nc.declare_dram_parameter("x", [NN, FA], F32, isOutput=False)
    posT_d = nc.declare_dram_parameter("posT", [3, NN], F32, isOutput=False)
    e1s_d = nc.declare_dram_parameter("e1s", [128, NE // 16], I16, isOutput=False)
    e2s_d = nc.declare_dram_parameter("e2s", [128, NE // 16], I16, isOutput=False)
    edst_d = nc.declare_dram_parameter("edst", [128, NE // 16], I16, isOutput=False)
    b32a_d = nc.declare_dram_parameter("blob32a", [128, CA], F32, isOutput=False)
    b32b_d = nc.declare_dram_parameter("blob32b", [128, CB], F32, isOutput=False)
    b16a_d = nc.declare_dram_parameter("blob16a", [128, CC], F16, isOutput=False)
    b16b_d = nc.declare_dram_parameter("blob16b", [128, CD], F16, isOutput=False)
    out_d = nc.declare_dram_parameter("out", [B, 1], F32, isOutput=True)

    tablex = nc.dram_tensor("tablex", [NN, 128], F16)
    table2q = nc.dram_tensor("table2q", [NN, 128], F16)
    table2rd = nc.dram_tensor("table2rd", [NN, 128], F16)
    r21_dr = nc.dram_tensor("r21_dr", [128, NE], F16)
    r22_dr = nc.dram_tensor("r22_dr", [128, NE], F16)
    cc_nch = {"bn11": 1, "bn12": 1, "bn13": 1, "bn21": 1, "bn22": 1,
              "bn23": 2, "bn31": 2, "bn32": 4}
    cc_in, cc_out = {}, {}
    for k, nch in cc_nch.items():
        cc_in[k] = nc.dram_tensor(f"ccin_{k}", [128, 2 * nch], F32)
        cc_out[k] = nc.dram_tensor(f"ccout_{k}", [NCORE, 128, 2 * nch], F32,
                                   addr_space="Shared")
    cc_in["bn33x"] = nc.dram_tensor("ccin_bn33x", [128, 80], F32)
    cc_out["bn33x"] = nc.dram_tensor("ccout_bn33x", [NCORE, 128, 80], F32,
                                     addr_space="Shared")
    RG = [list(range(NCORE))]

    with tile.TileContext(nc) as tc, ExitStack() as ctx:
        const = ctx.enter_context(tc.tile_pool(name="const", bufs=1))
        acts = ctx.enter_context(tc.tile_pool(name="acts", bufs=1))
        ring = ctx.enter_context(tc.tile_pool(name="ring", bufs=2))
        junkp = ctx.enter_context(tc.tile_pool(name="junk", bufs=2))
        statp = ctx.enter_context(tc.tile_pool(name="stats", bufs=1))
        small = ctx.enter_context(tc.tile_pool(name="small", bufs=2))
        psA = ctx.enter_context(tc.tile_pool(name="psA", bufs=3, space="PSUM"))
        psS = ctx.enter_context(tc.tile_pool(name="psS", bufs=2, space="PSUM"))

        b32a = const.tile([128, CA], F32)
        nc.sync.dma_start(out=b32a, in_=b32a_d[:, :])
        b16a = const.tile([128, CC], F16)
        nc.sync.dma_start(out=b16a, in_=b16a_d[:, :])

        def WA(name):
            c0, w, r = SA[name]
            return b32a[:r, c0:c0 + w]

        def WC(name):
            c0, w, r = SC[name]
            return b16a[:r, c0:c0 + w]

        ones16 = const.tile([1, NN], F16)
        nc.vector.memset(ones16, 1.0)
        eps_t = const.tile([128, 1], F32)
        nc.vector.memset(eps_t, EPS)
        pos4 = const.tile([4, NN], F16)
        agg1 = acts.tile([128, NN], F16, tag="agg1")
        agg2 = acts.tile([128, 2, NN], F16, tag="agg2")

        # ---------- pos normalize + tablex ----------
        with tc.tile_pool(name="p0", bufs=1) as P0:
            posT = P0.tile([3, NN], F32)
            nc.sync.dma_start(out=posT, in_=posT_d[:, :])
            posg = posT.rearrange("p (g n) -> p g n", g=NG)
            pmean = P0.tile([3, NG], F32)
            nc.vector.reduce_sum(pmean, posg, axis=AX.X)
            nc.vector.tensor_scalar_mul(pmean, pmean, 1.0 / PTS)
            posn = P0.tile([3, NN], F32)
            png = posn.rearrange("p (g n) -> p g n", g=NG)
            pm_b = bass.AP(tensor=pmean.tensor, offset=pmean.offset,
                           ap=[pmean.ap[0], pmean.ap[1], [0, PTS]])
            nc.vector.tensor_sub(png, posg, pm_b)
            am = P0.tile([3, NG], F32)
            nc.vector.tensor_reduce(am, png, axis=AX.X, op=ALU.max,
                                    apply_absolute_value=True)
            m3 = P0.tile([3, NG], F32)
            import concourse.bass_isa as bass_isa
            nc.gpsimd.partition_all_reduce(m3, am, 3, bass_isa.ReduceOp.max)
            sc3 = P0.tile([3, NG], F32)
            nc.vector.reciprocal(sc3, m3)
            nc.vector.tensor_scalar_mul(sc3, sc3, 0.999999)
            sc_b = bass.AP(tensor=sc3.tensor, offset=sc3.offset,
                           ap=[sc3.ap[0], sc3.ap[1], [0, PTS]])
            nc.vector.tensor_mul(png, png, sc_b)
            nc.vector.memset(pos4, 1.0)
            nc.vector.tensor_copy(pos4[0:3, :], posn)

            xall = P0.tile([128, 16, FA], F32)
            nc.sync.dma_start(out=xall, in_=bass.AP(
                tensor=xin[:, :].tensor, offset=0,
                ap=[[FA, 128], [128 * FA, 16], [1, FA]]))
            stg = P0.tile([128, 16, 128], F16)
            for ch in range(16):
                nc.vector.tensor_copy(stg[:, ch, 0:FA], xall[:, ch, :])
                pst = psS.tile([128, 4], F16, tag="ps_small")
                nc.tensor.transpose(pst, pos4[:, ch * 128:(ch + 1) * 128], WC("id4"))
                nc.scalar.copy(stg[:, ch, FA:FA + 4], pst)
                nc.vector.memset(stg[:, ch, FA + 4:128], 0.0)
            nc.sync.dma_start(out=bass.AP(
                tensor=tablex[:, :].tensor, offset=0,
                ap=[[128, 128], [128 * 128, 16], [1, 128]]), in_=stg)

        # ---------- helpers ----------
        def stat_sync(name, sA, qD, g_ap, b_ap, n_batch, pairfold, nch):
            st = statp.tile([128, 2 * nch], F32, tag=f"st_{name}")
            for c in range(nch):
                nc.vector.reduce_sum(st[:, 2 * c:2 * c + 1], sA[:, c, :], axis=AX.X)
                nc.vector.reduce_sum(st[:, 2 * c + 1:2 * c + 2], qD[:, c, :], axis=AX.X)
            nc.sync.dma_start(out=cc_in[name][:, :], in_=st)
            nc.gpsimd.collective_compute(
                "AllGather", ALU.bypass, ins=[cc_in[name][:, :]],
                outs=[cc_out[name][:, :, :]], replica_groups=RG)
            w = 2 * nch
            allst = statp.tile([128, NCORE, 2 * nch], F32, tag=f"all_{name}")
            nc.sync.dma_start(out=allst, in_=bass.AP(
                tensor=cc_out[name][:, :, :].tensor, offset=0,
                ap=[[w, 128], [128 * w, NCORE], [1, 2 * nch]]))
            tot = statp.tile([128, 2 * nch], F32, tag=f"tot_{name}")
            aswap = bass.AP(tensor=allst.tensor, offset=allst.offset,
                            ap=[allst.ap[0], [1, 2 * nch], [2 * nch, NCORE]])
            nc.vector.reduce_sum(tot, aswap, axis=AX.X)
            if pairfold:
                pf = psS.tile([128, 2 * nch], F32, tag="ps_small")
                nc.tensor.matmul(pf, WA("pairsum"), tot, start=True, stop=True)
                nc.scalar.copy(tot, pf)
            nc.vector.tensor_scalar_mul(tot, tot, 1.0 / n_batch)
            s_t = statp.tile([128, nch], F32, tag=f"s_{name}")
            t_t = statp.tile([128, nch], F32, tag=f"t_{name}")
            for c in range(nch):
                mn = tot[:, 2 * c:2 * c + 1]
                ex2 = tot[:, 2 * c + 1:2 * c + 2]
                var = small.tile([128, 1], F32, tag="var")
                nc.vector.tensor_mul(var, mn, mn)
                nc.vector.tensor_sub(var, ex2, var)
                lnv = small.tile([128, 1], F32, tag="lnv")
                nc.scalar.activation(lnv, var, AF.Ln, bias=eps_t)
                rstd = small.tile([128, 1], F32, tag="rstd")
                nc.scalar.activation(rstd, lnv, AF.Exp, scale=-0.5)
                nc.vector.tensor_mul(s_t[:, c:c + 1], g_ap[:, c:c + 1], rstd)
                ms = small.tile([128, 1], F32, tag="ms")
                nc.vector.tensor_mul(ms, mn, s_t[:, c:c + 1])
                nc.vector.tensor_sub(t_t[:, c:c + 1], b_ap[:, c:c + 1], ms)
            return s_t, t_t

        def fold_w(w_ap, s_t, kchunks, dout, tag):
            wf = acts.tile([128, kchunks, dout], F16, tag=tag)
            for k in range(kchunks):
                nc.vector.tensor_scalar_mul(
                    wf[:, k, :], w_ap[:, k * dout:(k + 1) * dout], s_t[:, k:k + 1])
            return wf

        def bias_row(t_t, w_ap, br_ap, kchunks, dout, ksz, tag):
            br = small.tile([1, dout], F16, tag=tag)
            nseg = (dout + 511) // 512
            for s in range(nseg):
                c0 = s * 512
                c1 = min(dout, c0 + 512)
                pb = psS.tile([1, 512], F32, tag="ps_small")
                for k in range(kchunks):
                    nc.tensor.matmul(pb[:, 0:c1 - c0],
                                     t_t[:ksz, k:k + 1],
                                     w_ap[:ksz, k * dout + c0:k * dout + c1],
                                     start=(k == 0), stop=(k == kchunks - 1))
                nc.vector.tensor_add(br[:, c0:c1], pb[:, 0:c1 - c0], br_ap[:, c0:c1])
            return br

        def slots(nch, nblk, tag):
            sA = statp.tile([128, nch, nblk], F32, tag=tag + "A")
            qD = statp.tile([128, nch, nblk], F32, tag=tag + "Q")
            return sA, qD

        def junk16():
            return junkp.tile([128, BLK], F16, tag="junk", name="junk")

        def tree_max(rg, out_ap):
            nn_ = BLK // DEG
            v = rg.rearrange("p (n w) -> p n w", n=nn_)
            t1 = ring.tile([128, nn_, 8], F16, tag="t1")
            nc.vector.tensor_max(t1, v[:, :, 0:8], v[:, :, 8:16])
            t2 = ring.tile([128, nn_, 4], F16, tag="t2")
            nc.vector.tensor_max(t2, t1[:, :, 0:4], t1[:, :, 4:8])
            t3 = ring.tile([128, nn_, 2], F16, tag="t3")
            nc.vector.tensor_max(t3, t2[:, :, 0:2], t2[:, :, 2:4])
            nc.vector.tensor_max(out_ap.rearrange("p (n w) -> p n w", n=nn_),
                                 t3[:, :, 0:1], t3[:, :, 1:2])

        # ============ conv1 ============
        with tc.tile_pool(name="p1", bufs=1) as P1, \
             tc.tile_pool(name="p1g", bufs=2) as P1G:
            e1s_t = P1.tile([128, NE // 16], I16)
            nc.sync.dma_start(out=e1s_t, in_=e1s_d[:, :])
            edst_t = P1.tile([128, NE // 16], I16)
            nc.sync.dma_start(out=edst_t, in_=edst_d[:, :])
            r11 = P1.tile([128, NE // 2], F16)
            r12 = P1.tile([128, NE // 2], F16)

            nblk11 = NE // (2 * BLK)        # 16
            sA11, qD11 = slots(1, nblk11, "s11")
            w1e = WC("w1ext")
            w1d = WC("w1dn")
            NBLK11E = NE // (2 * BLK)       # 16 blocks of 2048 edges
            for blk in range(NBLK11E):
                gx = P1G.tile([128, 1, 2 * BLK], F16, tag="gx", name="gx")
                gd = P1G.tile([128, 1, 2 * BLK], F16, tag="gd", name="gd")
                for gsub in range(2 * BLK // GCH):
                    col = (blk * 2 * BLK + gsub * GCH) // 16
                    nc.gpsimd.dma_gather(
                        out_ap=gx[:, :, gsub * GCH:(gsub + 1) * GCH],
                        in_ap=tablex[:, :],
                        idxs_ap=e1s_t[:, col:col + GCH // 16],
                        num_idxs=GCH, num_idxs_reg=GCH,
                        elem_size=128, transpose=True, queue_num=0)
                    nc.gpsimd.dma_gather(
                        out_ap=gd[:, :, gsub * GCH:(gsub + 1) * GCH],
                        in_ap=tablex[:, :],
                        idxs_ap=edst_t[:, col:col + GCH // 16],
                        num_idxs=GCH, num_idxs_reg=GCH,
                        elem_size=128, transpose=True, queue_num=0)
                ps = psA.tile([128, BLK], F32, tag="psA", name="ps")
                for q in range(2):
                    e0 = q * BLK
                    cps = ps[:, q * 512:(q + 1) * 512]
                    nc.tensor.matmul(cps[0:64, :], w1e, gx[:, 0, e0:e0 + 512],
                                     start=True, stop=False, tile_position=(0, 0))
                    nc.tensor.matmul(cps[0:64, :], w1d, gd[:, 0, e0:e0 + 512],
                                     start=False, stop=False, tile_position=(0, 0))
                    nc.tensor.matmul(cps[64:128, :], w1e,
                                     gx[:, 0, e0 + 512:e0 + 1024],
                                     start=True, stop=False, tile_position=(0, 64))
                    nc.tensor.matmul(cps[64:128, :], w1d,
                                     gd[:, 0, e0 + 512:e0 + 1024],
                                     start=False, stop=True, tile_position=(0, 64))
                rr = r11[:, blk * BLK:(blk + 1) * BLK]
                nc.scalar.activation(rr, ps, AF.Relu,
                                     accum_out=sA11[:, 0, blk:blk + 1])
                nc.vector.scalar_tensor_tensor(
                    out=junk16(), in0=rr, scalar=0.0, in1=rr,
                    op0=ALU.bypass, op1=ALU.mult,
                    accum_out=qD11[:, 0, blk:blk + 1])
            s11, t11 = stat_sync("bn11", sA11, qD11, WA("g11"), WA("be11"),
                                 NBAT_E, True, 1)
            w12f = fold_w(WA("w12d"), s11, 1, 64, "w12f")
            br12 = small.tile([1, 128], F16, tag="br12")
            pb12 = psS.tile([1, 512], F32, tag="ps_small")
            nc.tensor.matmul(pb12[:, 0:128], t11[0:64, 0:1], WA("w12s")[0:64, :],
                             start=True, stop=True)
            nc.vector.tensor_add(br12, pb12[:, 0:128], WA("b12r")[0:1, :])

            # L12
            sA12, qD12 = slots(1, nblk11, "s12")
            for blk in range(nblk11):
                ps = psA.tile([128, BLK], F32, tag="psA")
                for q in range(2):
                    cps = ps[:, q * 512:(q + 1) * 512]
                    c0 = blk * BLK + q * 512
                    rs = r11[:, c0:c0 + 512]
                    nc.tensor.matmul(cps[0:64, :], w12f[0:64, 0, :], rs[0:64, :],
                                     start=True, stop=False, tile_position=(0, 0))
                    nc.tensor.matmul(cps[64:128, :], w12f[64:128, 0, :],
                                     rs[64:128, :], start=True, stop=False,
                                     tile_position=(64, 64))
                    nc.tensor.matmul(cps, br12, ones16[:, 0:512],
                                     start=False, stop=True)
                rr = r12[:, blk * BLK:(blk + 1) * BLK]
                nc.scalar.activation(rr, ps, AF.Relu,
                                     accum_out=sA12[:, 0, blk:blk + 1])
                nc.vector.scalar_tensor_tensor(
                    out=junk16(), in0=rr, scalar=0.0, in1=rr,
                    op0=ALU.bypass, op1=ALU.mult, accum_out=qD12[:, 0, blk:blk + 1])
            s12, t12 = stat_sync("bn12", sA12, qD12, WA("g12"), WA("be12"),
                                 NBAT_E, True, 1)
            w13f = fold_w(WA("w13d"), s12, 1, 128, "w13f")
            br13 = bias_row(t12, WA("w13d"), WA("b13r"), 1, 128, 64, "br13")

            # L13: unpacked; aggregation
            nblk13 = NE // BLK              # 32
            sA13, qD13 = slots(1, nblk13, "s13")
            for blk in range(nblk13):
                ps = psA.tile([128, BLK], F32, tag="psA")
                for q in range(2):
                    g512 = 2 * blk + q
                    half = g512 % 2
                    col0 = (g512 // 4) * 1024 + ((g512 // 2) % 2) * 512
                    base = 64 * half
                    rs = r12[base:base + 64, col0:col0 + 512]
                    cps = ps[:, q * 512:(q + 1) * 512]
                    nc.tensor.matmul(cps, w13f[base:base + 64, 0, :], rs,
                                     start=True, stop=False, tile_position=(base, 0))
                    nc.tensor.matmul(cps, br13, ones16[:, 0:512],
                                     start=False, stop=True)
                rg = ring.tile([128, BLK], F16, tag="ring1")
                nc.scalar.activation(rg, ps, AF.Relu,
                                     accum_out=sA13[:, 0, blk:blk + 1])
                nc.vector.scalar_tensor_tensor(
                    out=junk16(), in0=rg, scalar=0.0, in1=rg,
                    op0=ALU.bypass, op1=ALU.mult, accum_out=qD13[:, 0, blk:blk + 1])
                nn_ = BLK // DEG
                tree_max(rg, agg1[:, blk * nn_:(blk + 1) * nn_])
            s13, t13 = stat_sync("bn13", sA13, qD13, WA("g13"), WA("be13"),
                                 NBAT_E, False, 1)

            # tables for conv2
            w2xf = fold_w(WA("w2x"), s13, 1, 128, "w2xf")
            brq2 = bias_row(t13, WA("w2x"), WA("b21r"), 1, 128, 128, "brq2")
            w2p4 = WC("w2p4")
            for ch in range(16):
                c0, c1 = ch * 128, (ch + 1) * 128
                ps = psS.tile([128, 128], F32, tag="ps_small")
                nc.tensor.matmul(ps, agg1[:, c0:c1], w2xf[:, 0, :],
                                 start=True, stop=False)
                nc.tensor.matmul(ps, pos4[:, c0:c1], w2p4, start=False, stop=False)
                nc.tensor.matmul(ps, ones16[:, c0:c1], brq2, start=False, stop=True)
                tq = small.tile([128, 128], F16, tag="tq")
                nc.scalar.copy(tq, ps)
                nc.sync.dma_start(out=bass.AP(
                    tensor=table2q[:, :].tensor, offset=ch * 128 * 128,
                    ap=[[128, 128], [1, 128]]), in_=tq)
                ps2 = psS.tile([128, 128], F32, tag="ps_small")
                nc.tensor.matmul(ps2, pos4[:, c0:c1], w2p4, start=True, stop=True)
                tr = small.tile([128, 128], F16, tag="tr")
                nc.scalar.copy(tr, ps2)
                nc.sync.dma_start(out=bass.AP(
                    tensor=table2rd[:, :].tensor, offset=ch * 128 * 128,
                    ap=[[128, 128], [1, 128]]), in_=tr)

        # ============ conv2 ============
        nblk2 = NE // BLK                   # 32
        with tc.tile_pool(name="p2", bufs=1) as P2, \
             tc.tile_pool(name="p2g", bufs=2) as P2G, \
             tc.tile_pool(name="p2s", bufs=3) as P2S:
            e2s_t = P2.tile([128, NE // 16], I16)
            nc.sync.dma_start(out=e2s_t, in_=e2s_d[:, :])
            edst_t2 = P2.tile([128, NE // 16], I16)
            nc.sync.dma_start(out=edst_t2, in_=edst_d[:, :])

            # L21: r21 = relu(q2[src] - rd2[dst]); spill to DRAM
            sA21, qD21 = slots(1, nblk2, "s21")
            for blk in range(nblk2):
                g2 = P2G.tile([128, 1, BLK], F16, tag="gx", name="g2")
                d2 = P2G.tile([128, 1, BLK], F16, tag="gd", name="d2")
                for gsub in range(BLK // GCH):
                    col = (blk * BLK + gsub * GCH) // 16
                    nc.gpsimd.dma_gather(
                        out_ap=g2[:, :, gsub * GCH:(gsub + 1) * GCH],
                        in_ap=table2q[:, :],
                        idxs_ap=e2s_t[:, col:col + GCH // 16],
                        num_idxs=GCH, num_idxs_reg=GCH,
                        elem_size=128, transpose=True, queue_num=0)
                    nc.gpsimd.dma_gather(
                        out_ap=d2[:, :, gsub * GCH:(gsub + 1) * GCH],
                        in_ap=table2rd[:, :],
                        idxs_ap=edst_t2[:, col:col + GCH // 16],
                        num_idxs=GCH, num_idxs_reg=GCH,
                        elem_size=128, transpose=True, queue_num=0)
                rr = P2S.tile([128, BLK], F16, tag="rblk", name="rr")
                sub = P2S.tile([128, BLK], F16, tag="sub", name="sub")
                nc.vector.tensor_sub(sub, g2[:, 0, :], d2[:, 0, :])
                nc.scalar.activation(rr, sub, AF.Relu,
                                     accum_out=sA21[:, 0, blk:blk + 1])
                nc.vector.scalar_tensor_tensor(
                    out=junk16(), in0=rr, scalar=0.0, in1=rr,
                    op0=ALU.bypass, op1=ALU.mult,
                    accum_out=qD21[:, 0, blk:blk + 1])
                nc.sync.dma_start(out=bass.AP(
                    tensor=r21_dr[:, :].tensor, offset=blk * BLK,
                    ap=[[NE, 128], [1, BLK]]), in_=rr)
            s21, t21 = stat_sync("bn21", sA21, qD21, WA("g21"), WA("be21"),
                                 NBAT_E, False, 1)
            w22f = fold_w(WA("w22"), s21, 1, 128, "w22f")
            br22 = bias_row(t21, WA("w22"), WA("b22r"), 1, 128, 128, "br22")

            # L22: 128->128; stream r21 in, spill r22 out
            sA22, qD22 = slots(1, nblk2, "s22")
            for blk in range(nblk2):
                rin = P2S.tile([128, BLK], F16, tag="rin")
                nc.sync.dma_start(out=rin, in_=bass.AP(
                    tensor=r21_dr[:, :].tensor, offset=blk * BLK,
                    ap=[[NE, 128], [1, BLK]]))
                ps = psA.tile([128, BLK], F32, tag="psA")
                for q in range(2):
                    cps = ps[:, q * 512:(q + 1) * 512]
                    nc.tensor.matmul(cps, w22f[:, 0, :], rin[:, q * 512:(q + 1) * 512],
                                     start=True, stop=False)
                    nc.tensor.matmul(cps, br22, ones16[:, 0:512],
                                     start=False, stop=True)
                rr = P2S.tile([128, BLK], F16, tag="rblk")
                nc.scalar.activation(rr, ps, AF.Relu,
                                     accum_out=sA22[:, 0, blk:blk + 1])
                nc.vector.scalar_tensor_tensor(
                    out=junk16(), in0=rr, scalar=0.0, in1=rr,
                    op0=ALU.bypass, op1=ALU.mult, accum_out=qD22[:, 0, blk:blk + 1])
                nc.sync.dma_start(out=bass.AP(
                    tensor=r22_dr[:, :].tensor, offset=blk * BLK,
                    ap=[[NE, 128], [1, BLK]]), in_=rr)
            s22, t22 = stat_sync("bn22", sA22, qD22, WA("g22"), WA("be22"),
                                 NBAT_E, False, 1)
            w23f = fold_w(WA("w23"), s22, 1, 256, "w23f")
            br23 = bias_row(t22, WA("w23"), WA("b23r"), 1, 256, 128, "br23")

            # L23: 128->256; stream r22 in (per M-chunk); aggregate
            sA23, qD23 = slots(2, nblk2, "s23")
            for mc in range(2):
                for blk in range(nblk2):
                    rin = P2S.tile([128, BLK], F16, tag="rin")
                    nc.sync.dma_start(out=rin, in_=bass.AP(
                        tensor=r22_dr[:, :].tensor, offset=blk * BLK,
                        ap=[[NE, 128], [1, BLK]]))
                    ps = psA.tile([128, BLK], F32, tag="psA")
                    for q in range(2):
                        cps = ps[:, q * 512:(q + 1) * 512]
                        nc.tensor.matmul(cps, w23f[:, 0, mc * 128:(mc + 1) * 128],
                                         rin[:, q * 512:(q + 1) * 512],
                                         start=True, stop=False)
                        nc.tensor.matmul(cps, br23[:, mc * 128:(mc + 1) * 128],
                                         ones16[:, 0:512], start=False, stop=True)
                    rg = ring.tile([128, BLK], F16, tag="ring1")
                    nc.scalar.activation(rg, ps, AF.Relu,
                                         accum_out=sA23[:, mc, blk:blk + 1])
                    nc.vector.scalar_tensor_tensor(
                        out=junk16(), in0=rg, scalar=0.0, in1=rg,
                        op0=ALU.bypass, op1=ALU.mult,
                        accum_out=qD23[:, mc, blk:blk + 1])
                    nn_ = BLK // DEG
                    tree_max(rg, agg2[:, mc, blk * nn_:(blk + 1) * nn_])
            s23, t23 = stat_sync("bn23", sA23, qD23, WA("g23"), WA("be23"),
                                 NBAT_E, False, 2)

        # ============ mlp3 ============
        nblkN = NN // BLK                   # 2
        with tc.tile_pool(name="p3", bufs=1) as P3:
            b32b = P3.tile([128, CB], F32)
            nc.sync.dma_start(out=b32b, in_=b32b_d[:, :])

            def WB(name):
                c0, w, r = SB[name]
                return b32b[:r, c0:c0 + w]

            w3af = fold_w(WB("w3a"), s23, 2, 256, "w3af")
            br31 = bias_row(t23, WB("w3a"), WB("b31r"), 2, 256, 128, "br31")
            w3p4 = WC("w3p4")
            r31 = P3.tile([128, 2, NN], F16)
            sA31, qD31 = slots(2, nblkN, "s31")
            for mc in range(2):
                for blk in range(nblkN):
                    ps = psA.tile([128, BLK], F32, tag="psA")
                    for q in range(2):
                        cs = blk * BLK + q * 512
                        cps = ps[:, q * 512:(q + 1) * 512]
                        nc.tensor.matmul(cps, w3af[:, 0, mc * 128:(mc + 1) * 128],
                                         agg2[:, 0, cs:cs + 512], start=True, stop=False)
                        nc.tensor.matmul(cps, w3af[:, 1, mc * 128:(mc + 1) * 128],
                                         agg2[:, 1, cs:cs + 512], start=False, stop=False)
                        nc.tensor.matmul(cps, w3p4[:, mc * 128:(mc + 1) * 128],
                                         pos4[:, cs:cs + 512], start=False, stop=False)
                        nc.tensor.matmul(cps, br31[:, mc * 128:(mc + 1) * 128],
                                         ones16[:, 0:512], start=False, stop=True)
                    rr = r31[:, mc, blk * BLK:(blk + 1) * BLK]
                    nc.scalar.activation(rr, ps, AF.Relu,
                                         accum_out=sA31[:, mc, blk:blk + 1])
                    nc.vector.scalar_tensor_tensor(
                        out=junk16(), in0=rr, scalar=0.0, in1=rr,
                        op0=ALU.bypass, op1=ALU.mult,
                        accum_out=qD31[:, mc, blk:blk + 1])
            s31, t31 = stat_sync("bn31", sA31, qD31, WB("g31"), WB("be31"),
                                 NBAT_N, False, 2)

            w3bf = fold_w(WB("w3b"), s31, 2, 512, "w3bf")
            br32 = bias_row(t31, WB("w3b"), WB("b32r"), 2, 512, 128, "br32")
            r32 = P3.tile([128, 4, NN], F16)
            sA32, qD32 = slots(4, nblkN, "s32")
            for mc in range(4):
                for blk in range(nblkN):
                    ps = psA.tile([128, BLK], F32, tag="psA")
                    for q in range(2):
                        cs = blk * BLK + q * 512
                        cps = ps[:, q * 512:(q + 1) * 512]
                        for kc in range(2):
                            nc.tensor.matmul(cps, w3bf[:, kc, mc * 128:(mc + 1) * 128],
                                             r31[:, kc, cs:cs + 512],
                                             start=(kc == 0), stop=False)
                        nc.tensor.matmul(cps, br32[:, mc * 128:(mc + 1) * 128],
                                         ones16[:, 0:512], start=False, stop=True)
                    rr = r32[:, mc, blk * BLK:(blk + 1) * BLK]
                    nc.scalar.activation(rr, ps, AF.Relu,
                                         accum_out=sA32[:, mc, blk:blk + 1])
                    nc.vector.scalar_tensor_tensor(
                        out=junk16(), in0=rr, scalar=0.0, in1=rr,
                        op0=ALU.bypass, op1=ALU.mult,
                        accum_out=qD32[:, mc, blk:blk + 1])
            s32, t32 = stat_sync("bn32", sA32, qD32, WB("g32"), WB("be32"),
                                 NBAT_N, False, 4)

            w3cf = fold_w(WB("w3c"), s32, 4, 1024, "w3cf")
            br33 = bias_row(t32, WB("w3c"), WB("b33r"), 4, 1024, 128, "br33")
            xgp = statp.tile([128, 8, NG], F32, tag="xgp")
            sA33, qD33 = slots(8, nblkN, "s33")
            for mc in range(8):
                for blk in range(nblkN):
                    ps = psA.tile([128, BLK], F32, tag="psA")
                    for q in range(2):
                        cs = blk * BLK + q * 512
                        cps = ps[:, q * 512:(q + 1) * 512]
                        for kc in range(4):
                            nc.tensor.matmul(cps, w3cf[:, kc, mc * 128:(mc + 1) * 128],
                                             r32[:, kc, cs:cs + 512],
                                             start=(kc == 0), stop=False)
                        nc.tensor.matmul(cps, br33[:, mc * 128:(mc + 1) * 128],
                                         ones16[:, 0:512], start=False, stop=True)
                    rg = ring.tile([128, BLK], F16, tag="ring1")
                    nc.scalar.activation(rg, ps, AF.Relu,
                                         accum_out=sA33[:, mc, blk:blk + 1])
                    nc.vector.scalar_tensor_tensor(
                        out=junk16(), in0=rg, scalar=0.0, in1=rg,
                        op0=ALU.bypass, op1=ALU.mult,
                        accum_out=qD33[:, mc, blk:blk + 1])
                    ngb = BLK // PTS
                    nc.vector.reduce_max(
                        xgp[:, mc, blk * ngb:(blk + 1) * ngb],
                        rg.rearrange("p (g n) -> p g n", g=ngb), axis=AX.X)

            # batched bn33 stats + pooled features AllGather
            stf = statp.tile([128, 80], F32, tag="stf")
            for c in range(8):
                nc.vector.reduce_sum(stf[:, 2 * c:2 * c + 1], sA33[:, c, :], axis=AX.X)
                nc.vector.reduce_sum(stf[:, 2 * c + 1:2 * c + 2], qD33[:, c, :],
                                     axis=AX.X)
            nc.vector.tensor_copy(stf[:, 16:80], xgp.rearrange("p a b -> p (a b)"))
            nc.sync.dma_start(out=cc_in["bn33x"][:, :], in_=stf)
            nc.gpsimd.collective_compute(
                "AllGather", ALU.bypass, ins=[cc_in["bn33x"][:, :]],
                outs=[cc_out["bn33x"][:, :, :]], replica_groups=RG)
            w = 80
            allst = statp.tile([128, NCORE, 16], F32, tag="all33")
            nc.sync.dma_start(out=allst, in_=bass.AP(
                tensor=cc_out["bn33x"][:, :, :].tensor, offset=0,
                ap=[[w, 128], [128 * w, NCORE], [1, 16]]))
            tot33 = statp.tile([128, 16], F32, tag="tot33")
            aswap33 = bass.AP(tensor=allst.tensor, offset=allst.offset,
                              ap=[allst.ap[0], [1, 16], [16, NCORE]])
            nc.vector.reduce_sum(tot33, aswap33, axis=AX.X)
            nc.vector.tensor_scalar_mul(tot33, tot33, 1.0 / NBAT_N)
            s33 = statp.tile([128, 8], F32, tag="s33")
            t33 = statp.tile([128, 8], F32, tag="t33")
            for c in range(8):
                mn = tot33[:, 2 * c:2 * c + 1]
                ex2 = tot33[:, 2 * c + 1:2 * c + 2]
                var = small.tile([128, 1], F32, tag="var")
                nc.vector.tensor_mul(var, mn, mn)
                nc.vector.tensor_sub(var, ex2, var)
                lnv = small.tile([128, 1], F32, tag="lnv")
                nc.scalar.activation(lnv, var, AF.Ln, bias=eps_t)
                rstd = small.tile([128, 1], F32, tag="rstd")
                nc.scalar.activation(rstd, lnv, AF.Exp, scale=-0.5)
                nc.vector.tensor_mul(s33[:, c:c + 1], WB("g33")[:, c:c + 1], rstd)
                ms = small.tile([128, 1], F32, tag="ms")
                nc.vector.tensor_mul(ms, mn, s33[:, c:c + 1])
                nc.vector.tensor_sub(t33[:, c:c + 1], WB("be33")[:, c:c + 1], ms)
            t33h = statp.tile([128, 8], F16, tag="t33h")
            nc.vector.tensor_copy(t33h, t33)

            # ============ head (replicated) ============
            b16b = P3.tile([128, CD], F16)
            nc.sync.dma_start(out=b16b, in_=b16b_d[:, :])

            def WD(name):
                c0, w_, r = SD[name]
                return b16b[:r, c0:c0 + w_]

            xga = statp.tile([128, NCORE, 64], F32, tag="xgar")
            nc.sync.dma_start(out=xga, in_=bass.AP(
                tensor=cc_out["bn33x"][:, :, :].tensor, offset=16,
                ap=[[w, 128], [128 * w, NCORE], [1, 64]]))
            xg = statp.tile([128, 8, B], F16, tag="xg")
            for kc in range(8):
                nc.scalar.activation(
                    xg[:, kc, :].rearrange("p (a b) -> p a b", a=NCORE),
                    xga[:, :, kc * NG:(kc + 1) * NG],
                    AF.Relu, scale=s33[:, kc:kc + 1])

            lin1h = WD("lin1")
            lin2h = WD("lin2")
            regh = WD("reg")
            o1 = statp.tile([128, 4, B], F16, tag="o1")
            for m in range(4):
                # t33 contribution to pre-BN mean shift: z = xg@W + t33@W (+b);
                # BN removes constant shifts, so only s33-scaled xg matters for
                # the centered value -- but t33@W shifts the mean, variance
                # unchanged; bias b absorbed by BN. Compute column shift:
                ps = psS.tile([128, B], F32, tag="ps_small")
                for kc in range(8):
                    nc.tensor.matmul(ps, lin1h[:, kc * 512 + m * 128:kc * 512 + (m + 1) * 128],
                                     xg[:, kc, :], start=(kc == 0), stop=(kc == 7))
                bs = small.tile([128, 6], F32, tag="bs")
                nc.vector.bn_stats(bs, ps)
                mv = small.tile([128, 2], F32, tag="mv")
                nc.vector.bn_aggr(mv, bs)
                lnv = small.tile([128, 1], F32, tag="lnv")
                nc.scalar.activation(lnv, mv[:, 1:2], AF.Ln, bias=eps_t)
                rstd = small.tile([128, 1], F32, tag="rstd")
                nc.scalar.activation(rstd, lnv, AF.Exp, scale=-0.5)
                sh = small.tile([128, 1], F32, tag="sh")
                nc.vector.tensor_mul(sh, WB("gh1")[:, m:m + 1], rstd)
                th = small.tile([128, 1], F32, tag="th")
                nc.vector.tensor_mul(th, mv[:, 0:1], sh)
                nc.vector.tensor_sub(th, WB("bh1")[:, m:m + 1], th)
                nc.scalar.activation(o1[:, m, :], ps, AF.Relu, scale=sh, bias=th)

            o2 = statp.tile([128, 4, B], F16, tag="o2")
            for m in range(4):
                ps = psS.tile([128, B], F32, tag="ps_small")
                for kc in range(4):
                    nc.tensor.matmul(ps, lin2h[:, kc * 512 + m * 128:kc * 512 + (m + 1) * 128],
                                     o1[:, kc, :], start=(kc == 0), stop=(kc == 3))
                bs = small.tile([128, 6], F32, tag="bs")
                nc.vector.bn_stats(bs, ps)
                mv = small.tile([128, 2], F32, tag="mv")
                nc.vector.bn_aggr(mv, bs)
                lnv = small.tile([128, 1], F32, tag="lnv")
                nc.scalar.activation(lnv, mv[:, 1:2], AF.Ln, bias=eps_t)
                rstd = small.tile([128, 1], F32, tag="rstd")
                nc.scalar.activation(rstd, lnv, AF.Exp, scale=-0.5)
                sh = small.tile([128, 1], F32, tag="sh")
                nc.vector.tensor_mul(sh, WB("gh2")[:, m:m + 1], rstd)
                th = small.tile([128, 1], F32, tag="th")
                nc.vector.tensor_mul(th, mv[:, 0:1], sh)
                nc.vector.tensor_sub(th, WB("bh2")[:, m:m + 1], th)
                nc.scalar.activation(o2[:, m, :], ps, AF.Relu, scale=sh, bias=th)

            pso = psS.tile([1, B], F32, tag="ps_small")
            for kc in range(4):
                nc.tensor.matmul(pso, regh[:, kc:kc + 1], o2[:, kc, :],
                                 start=(kc == 0), stop=(kc == 3))
            outt = small.tile([1, B], F32, tag="outt")
            nc.scalar.activation(outt, pso, AF.Identity, bias=WB("regb")[0:1, 0:1])
            nc.sync.dma_start(out=bass.AP(tensor=out_d[:, :].tensor, offset=0,
                                          ap=[[0, 1], [1, B]]), in_=outt)

    nc.compile()
    return nc


_CACHE = {}


def kernel(**inputs):
    in_maps, shapes = _host_prep(inputs)
    if "nc" not in _CACHE:
        nc = bacc.Bacc()
        build_kernel(nc, shapes)
        _CACHE["nc"] = nc
    res = run_bass_kernel_spmd(nc := _CACHE["nc"], in_maps, list(range(NCORE)))
    return np.ascontiguousarray(np.asarray(res.results[0]["out"], np.float32))


# revision 15
# speedup vs baseline: 1.0938x; 1.0938x over previous
"""CrystalEncoder Trainium2 kernel: 8-way data-parallel over graphs.

Per core: 8 graphs = 2048 nodes, 32768 edges per edge set.
  - node feature tables in HBM (fp16, 128-col rows); edge gathers via
    dma_gather(transpose=True) -> [feat(part), edge(free)] fp16 tiles
  - MLP layers as fp16 matmuls (fp32 psum), feature-on-partition
  - BatchNorm (training-mode batch stats) folded into the next layer's
    weights; per-layer stats: ACT relu+accum (sum) + DVE STT (sumsq);
    cross-core AllGather of tiny [128, 2*nch] partials
  - segment-max aggregation commuted past relu/BN (monotone) -> window
    max trees on DVE
  - pooled [B,1024] features AllGather'ed, head replicated on all cores
"""

import numpy as np
from contextlib import ExitStack

import concourse.bass as bass
import concourse.bacc as bacc
import concourse.tile as tile
from concourse import mybir
from concourse.bass_utils import run_bass_kernel_spmd

F16 = mybir.dt.float16
F32 = mybir.dt.float32
I16 = mybir.dt.int16
AF = mybir.ActivationFunctionType
ALU = mybir.AluOpType
AX = mybir.AxisListType

NCORE = 8
B = 64
PTS = 256
NTOTN = B * PTS       # 16384
DEG = 16
FA = 98
NG = B // NCORE       # 8 graphs per core
NN = NG * PTS         # 2048 nodes per core
NE = NN * DEG         # 32768 edges per core
EPS = 1e-5
NBAT_E = float(NTOTN * DEG)
NBAT_N = float(NTOTN)

BLK = 1024            # psum block columns (2 banks)
GCH = 512             # gather chunk (idxs; SWDGE desc carveout is 1024)


def _wrap_idx(ix):
    """dma_gather index layout: idx i at [i%16, i//16], tiled to 128 parts."""
    t = ix.astype(np.int16).reshape(-1, 16).T
    return np.tile(t, (8, 1)).copy()


class _Blob:
    def __init__(self, dtype):
        self.dtype = dtype
        self.cols = []
        self.slices = {}
        self.c = 0

    def add(self, name, arr):
        a = np.asarray(arr, dtype=self.dtype)
        assert a.ndim == 2 and a.shape[0] <= 128, (name, a.shape)
        p = np.zeros((128, a.shape[1]), dtype=self.dtype)
        p[: a.shape[0], :] = a
        self.cols.append(p)
        self.slices[name] = (self.c, a.shape[1], a.shape[0])
        self.c += a.shape[1]

    def build(self):
        return np.concatenate(self.cols, axis=1)


def _host_prep(inputs):
    x = np.asarray(inputs["x"], np.float32)
    pos = np.asarray(inputs["pos"], np.float32)
    e1src = np.asarray(inputs["e1_src"], np.int32)
    e2src = np.asarray(inputs["e2_src"], np.int32)

    def WBGE(layer):
        return tuple(np.asarray(a, np.float32) for a in layer)

    m1 = [WBGE(l) for l in inputs["mlp1"]]
    m2 = [WBGE(l) for l in inputs["mlp2"]]
    m3 = [WBGE(l) for l in inputs["mlp3"]]
    L1W = np.asarray(inputs["lin1"][0], np.float32)
    L2W = np.asarray(inputs["lin2"][0], np.float32)
    RW = np.asarray(inputs["reg"][0], np.float32)
    Rb = np.asarray(inputs["reg"][1], np.float32)
    gh1, bh1 = (np.asarray(a, np.float32) for a in inputs["bn1"])
    gh2, bh2 = (np.asarray(a, np.float32) for a in inputs["bn2"])

    dup = lambda v: np.concatenate([v, v])
    col = lambda v: v.reshape(-1, 1)
    row = lambda v: v.reshape(1, -1)
    chcol = lambda v, nch: v.reshape(nch, 128).T

    ba = _Blob(np.float32)          # conv-phase f32 blob
    ps = np.zeros((128, 128), np.float32)
    for k in range(128):
        ps[k, k % 64] = 1.0
        ps[k, k % 64 + 64] = 1.0
    ba.add("pairsum", ps)
    ba.add("w12d", np.tile(m1[1][0], (2, 1)))
    ba.add("w12s", np.concatenate([m1[1][0], m1[1][0]], 1))
    ba.add("w13d", np.tile(m1[2][0], (2, 1)))
    ba.add("w2x", m2[0][0][:128])
    ba.add("w22", m2[1][0])
    ba.add("w23", m2[2][0])
    ba.add("b12r", row(dup(m1[1][1])))
    ba.add("b13r", row(m1[2][1]))
    ba.add("b21r", row(m2[0][1]))
    ba.add("b22r", row(m2[1][1]))
    ba.add("b23r", row(m2[2][1]))
    ba.add("g11", col(dup(m1[0][2]))); ba.add("be11", col(dup(m1[0][3])))
    ba.add("g12", col(dup(m1[1][2]))); ba.add("be12", col(dup(m1[1][3])))
    ba.add("g13", col(m1[2][2])); ba.add("be13", col(m1[2][3]))
    ba.add("g21", col(m2[0][2])); ba.add("be21", col(m2[0][3]))
    ba.add("g22", col(m2[1][2])); ba.add("be22", col(m2[1][3]))
    ba.add("g23", chcol(m2[2][2], 2)); ba.add("be23", chcol(m2[2][3], 2))
    blob32a = ba.build()

    bb = _Blob(np.float32)          # mlp3/head-phase f32 blob
    bb.add("w3a", np.concatenate([m3[0][0][0:128], m3[0][0][128:256]], 1))
    bb.add("w3b", np.concatenate([m3[1][0][0:128], m3[1][0][128:256]], 1))
    bb.add("w3c", np.concatenate([m3[2][0][k * 128:(k + 1) * 128] for k in range(4)], 1))
    bb.add("b31r", row(m3[0][1]))
    bb.add("b32r", row(m3[1][1]))
    bb.add("b33r", row(m3[2][1]))
    bb.add("g31", chcol(m3[0][2], 2)); bb.add("be31", chcol(m3[0][3], 2))
    bb.add("g32", chcol(m3[1][2], 4)); bb.add("be32", chcol(m3[1][3], 4))
    bb.add("g33", chcol(m3[2][2], 8)); bb.add("be33", chcol(m3[2][3], 8))
    bb.add("gh1", chcol(gh1, 4)); bb.add("bh1", chcol(bh1, 4))
    bb.add("gh2", chcol(gh2, 4)); bb.add("bh2", chcol(bh2, 4))
    bb.add("regb", np.array([[float(Rb[0])]], np.float32))
    blob32b = bb.build()

    bc = _Blob(np.float16)          # conv-phase fp16 blob
    w1ext = np.zeros((128, 64), np.float32)
    w1ext[0:101] = m1[0][0]
    w1ext[101] = m1[0][1]
    bc.add("w1ext", w1ext)
    w1dn = np.zeros((128, 64), np.float32)
    w1dn[98:101] = -m1[0][0][98:101]
    bc.add("w1dn", w1dn)
    w2p4 = np.zeros((4, 128), np.float32)
    w2p4[0:3] = m2[0][0][128:131]
    bc.add("w2p4", w2p4)
    w3p4 = np.zeros((4, 256), np.float32)
    w3p4[0:3] = m3[0][0][256:259]
    bc.add("w3p4", w3p4)
    bc.add("id4", np.eye(4, dtype=np.float32))
    blob16a = bc.build()

    bd = _Blob(np.float16)          # head fp16 blob
    bd.add("lin1", np.concatenate([L1W[k * 128:(k + 1) * 128] for k in range(8)], 1))
    bd.add("lin2", np.concatenate([L2W[k * 128:(k + 1) * 128] for k in range(4)], 1))
    bd.add("reg", np.concatenate([RW[k * 128:(k + 1) * 128] for k in range(4)], 1))
    blob16b = bd.build()

    dst = _wrap_idx(np.arange(NE, dtype=np.int32) // DEG)
    in_maps = []
    for c in range(NCORE):
        ns = c * NN
        es = c * NE
        in_maps.append({
            "x": np.ascontiguousarray(x[ns:ns + NN]),
            "posT": np.ascontiguousarray(pos[ns:ns + NN].T),
            "e1s": _wrap_idx(e1src[es:es + NE] - ns),
            "e2s": _wrap_idx(e2src[es:es + NE] - ns),
            "edst": dst,
            "blob32a": blob32a,
            "blob32b": blob32b,
            "blob16a": blob16a,
            "blob16b": blob16b,
        })
    shapes = (ba.slices, bb.slices, bc.slices, bd.slices,
              blob32a.shape[1], blob32b.shape[1], blob16a.shape[1], blob16b.shape[1])
    return in_maps, shapes


def build_kernel(nc, shapes):
    SA, SB, SC, SD, CA, CB, CC, CD = shapes
    xin = nc.declare_dram_parameter("x", [NN, FA], F32, isOutput=False)
    posT_d = nc.declare_dram_parameter("posT", [3, NN], F32, isOutput=False)
    e1s_d = nc.declare_dram_parameter("e1s", [128, NE // 16], I16, isOutput=False)
    e2s_d = nc.declare_dram_parameter("e2s", [128, NE // 16], I16, isOutput=False)
    edst_d = nc.declare_dram_parameter("edst", [128, NE // 16], I16, isOutput=False)
    b32a_d = nc.declare_dram_parameter("blob32a", [128, CA], F32, isOutput=False)
    b32b_d = nc.declare_dram_parameter("blob32b", [128, CB], F32, isOutput=False)
    b16a_d = nc.declare_dram_parameter("blob16a", [128, CC], F16, isOutput=False)
    b16b_d = nc.declare_dram_parameter("blob16b", [128, CD], F16, isOutput=False)
    out_d = nc.declare_dram_parameter("out", [B, 1], F32, isOutput=True)

    tablex = nc.dram_tensor("tablex", [NN, 128], F16)
    table2q = nc.dram_tensor("table2q", [NN, 128], F16)
    r21_dr = nc.dram_tensor("r21_dr", [128, NE], F16)
    r22_dr = nc.dram_tensor("r22_dr", [128, NE], F16)
    cc_nch = {"bn11": 1, "bn12": 1, "bn13": 1, "bn21": 1, "bn22": 1,
              "bn23": 2, "bn31": 2, "bn32": 4}
    cc_in, cc_out = {}, {}
    for k, nch in cc_nch.items():
        cc_in[k] = nc.dram_tensor(f"ccin_{k}", [128, 2 * nch], F32)
        cc_out[k] = nc.dram_tensor(f"ccout_{k}", [NCORE, 128, 2 * nch], F32,
                                   addr_space="Shared")
    cc_in["bn33x"] = nc.dram_tensor("ccin_bn33x", [128, 80], F32)
    cc_out["bn33x"] = nc.dram_tensor("ccout_bn33x", [NCORE, 128, 80], F32,
                                     addr_space="Shared")
    RG = [list(range(NCORE))]

    with tile.TileContext(nc) as tc, ExitStack() as ctx:
        const = ctx.enter_context(tc.tile_pool(name="const", bufs=1))
        acts = ctx.enter_context(tc.tile_pool(name="acts", bufs=1))
        ring = ctx.enter_context(tc.tile_pool(name="ring", bufs=2))
        junkp = ctx.enter_context(tc.tile_pool(name="junk", bufs=2))
        statp = ctx.enter_context(tc.tile_pool(name="stats", bufs=1))
        small = ctx.enter_context(tc.tile_pool(name="small", bufs=2))
        psA = ctx.enter_context(tc.tile_pool(name="psA", bufs=3, space="PSUM"))
        psS = ctx.enter_context(tc.tile_pool(name="psS", bufs=2, space="PSUM"))

        b32a = const.tile([128, CA], F32)
        nc.sync.dma_start(out=b32a, in_=b32a_d[:, :])
        b16a = const.tile([128, CC], F16)
        nc.sync.dma_start(out=b16a, in_=b16a_d[:, :])

        def WA(name):
            c0, w, r = SA[name]
            return b32a[:r, c0:c0 + w]

        def WC(name):
            c0, w, r = SC[name]
            return b16a[:r, c0:c0 + w]

        ones16 = const.tile([1, NN], F16)
        nc.vector.memset(ones16, 1.0)
        eps_t = const.tile([128, 1], F32)
        nc.vector.memset(eps_t, EPS)
        pos4 = const.tile([4, NN], F16)
        agg1 = acts.tile([128, NN], F16, tag="agg1")
        agg2 = acts.tile([128, 2, NN], F16, tag="agg2")
        rd2f = acts.tile([128, NN], F16, tag="rd2f")

        # ---------- pos normalize + tablex ----------
        with tc.tile_pool(name="p0", bufs=1) as P0:
            posT = P0.tile([3, NN], F32)
            nc.sync.dma_start(out=posT, in_=posT_d[:, :])
            posg = posT.rearrange("p (g n) -> p g n", g=NG)
            pmean = P0.tile([3, NG], F32)
            nc.vector.reduce_sum(pmean, posg, axis=AX.X)
            nc.vector.tensor_scalar_mul(pmean, pmean, 1.0 / PTS)
            posn = P0.tile([3, NN], F32)
            png = posn.rearrange("p (g n) -> p g n", g=NG)
            pm_b = bass.AP(tensor=pmean.tensor, offset=pmean.offset,
                           ap=[pmean.ap[0], pmean.ap[1], [0, PTS]])
            nc.vector.tensor_sub(png, posg, pm_b)
            am = P0.tile([3, NG], F32)
            nc.vector.tensor_reduce(am, png, axis=AX.X, op=ALU.max,
                                    apply_absolute_value=True)
            m3 = P0.tile([3, NG], F32)
            import concourse.bass_isa as bass_isa
            nc.gpsimd.partition_all_reduce(m3, am, 3, bass_isa.ReduceOp.max)
            sc3 = P0.tile([3, NG], F32)
            nc.vector.reciprocal(sc3, m3)
            nc.vector.tensor_scalar_mul(sc3, sc3, 0.999999)
            sc_b = bass.AP(tensor=sc3.tensor, offset=sc3.offset,
                           ap=[sc3.ap[0], sc3.ap[1], [0, PTS]])
            nc.vector.tensor_mul(png, png, sc_b)
            nc.vector.memset(pos4, 1.0)
            nc.vector.tensor_copy(pos4[0:3, :], posn)

            xall = P0.tile([128, 16, FA], F32)
            nc.sync.dma_start(out=xall, in_=bass.AP(
                tensor=xin[:, :].tensor, offset=0,
                ap=[[FA, 128], [128 * FA, 16], [1, FA]]))
            stg = P0.tile([128, 16, 128], F16)
            for ch in range(16):
                nc.vector.tensor_copy(stg[:, ch, 0:FA], xall[:, ch, :])
                pst = psS.tile([128, 4], F16, tag="ps_small")
                nc.tensor.transpose(pst, pos4[:, ch * 128:(ch + 1) * 128], WC("id4"))
                nc.scalar.copy(stg[:, ch, FA:FA + 4], pst)
                nc.vector.memset(stg[:, ch, FA + 4:128], 0.0)
            nc.sync.dma_start(out=bass.AP(
                tensor=tablex[:, :].tensor, offset=0,
                ap=[[128, 128], [128 * 128, 16], [1, 128]]), in_=stg)

        # ---------- helpers ----------
        def stat_sync(name, sA, qD, g_ap, b_ap, n_batch, pairfold, nch):
            st = statp.tile([128, 2 * nch], F32, tag=f"st_{name}")
            for c in range(nch):
                nc.vector.reduce_sum(st[:, 2 * c:2 * c + 1], sA[:, c, :], axis=AX.X)
                nc.vector.reduce_sum(st[:, 2 * c + 1:2 * c + 2], qD[:, c, :], axis=AX.X)
            nc.sync.dma_start(out=cc_in[name][:, :], in_=st)
            nc.gpsimd.collective_compute(
                "AllGather", ALU.bypass, ins=[cc_in[name][:, :]],
                outs=[cc_out[name][:, :, :]], replica_groups=RG)
            w = 2 * nch
            allst = statp.tile([128, NCORE, 2 * nch], F32, tag=f"all_{name}")
            nc.sync.dma_start(out=allst, in_=bass.AP(
                tensor=cc_out[name][:, :, :].tensor, offset=0,
                ap=[[w, 128], [128 * w, NCORE], [1, 2 * nch]]))
            tot = statp.tile([128, 2 * nch], F32, tag=f"tot_{name}")
            aswap = bass.AP(tensor=allst.tensor, offset=allst.offset,
                            ap=[allst.ap[0], [1, 2 * nch], [2 * nch, NCORE]])
            nc.vector.reduce_sum(tot, aswap, axis=AX.X)
            if pairfold:
                pf = psS.tile([128, 2 * nch], F32, tag="ps_small")
                nc.tensor.matmul(pf, WA("pairsum"), tot, start=True, stop=True)
                nc.scalar.copy(tot, pf)
            nc.vector.tensor_scalar_mul(tot, tot, 1.0 / n_batch)
            s_t = statp.tile([128, nch], F32, tag=f"s_{name}")
            t_t = statp.tile([128, nch], F32, tag=f"t_{name}")
            for c in range(nch):
                mn = tot[:, 2 * c:2 * c + 1]
                ex2 = tot[:, 2 * c + 1:2 * c + 2]
                var = small.tile([128, 1], F32, tag="var")
                nc.vector.tensor_mul(var, mn, mn)
                nc.vector.tensor_sub(var, ex2, var)
                lnv = small.tile([128, 1], F32, tag="lnv")
                nc.scalar.activation(lnv, var, AF.Ln, bias=eps_t)
                rstd = small.tile([128, 1], F32, tag="rstd")
                nc.scalar.activation(rstd, lnv, AF.Exp, scale=-0.5)
                nc.vector.tensor_mul(s_t[:, c:c + 1], g_ap[:, c:c + 1], rstd)
                ms = small.tile([128, 1], F32, tag="ms")
                nc.vector.tensor_mul(ms, mn, s_t[:, c:c + 1])
                nc.vector.tensor_sub(t_t[:, c:c + 1], b_ap[:, c:c + 1], ms)
            return s_t, t_t

        def fold_w(w_ap, s_t, kchunks, dout, tag):
            wf = acts.tile([128, kchunks, dout], F16, tag=tag)
            for k in range(kchunks):
                nc.vector.tensor_scalar_mul(
                    wf[:, k, :], w_ap[:, k * dout:(k + 1) * dout], s_t[:, k:k + 1])
            return wf

        def bias_row(t_t, w_ap, br_ap, kchunks, dout, ksz, tag):
            br = small.tile([1, dout], F16, tag=tag)
            nseg = (dout + 511) // 512
            for s in range(nseg):
                c0 = s * 512
                c1 = min(dout, c0 + 512)
                pb = psS.tile([1, 512], F32, tag="ps_small")
                for k in range(kchunks):
                    nc.tensor.matmul(pb[:, 0:c1 - c0],
                                     t_t[:ksz, k:k + 1],
                                     w_ap[:ksz, k * dout + c0:k * dout + c1],
                                     start=(k == 0), stop=(k == kchunks - 1))
                nc.vector.tensor_add(br[:, c0:c1], pb[:, 0:c1 - c0], br_ap[:, c0:c1])
            return br

        def slots(nch, nblk, tag):
            sA = statp.tile([128, nch, nblk], F32, tag=tag + "A")
            qD = statp.tile([128, nch, nblk], F32, tag=tag + "Q")
            return sA, qD

        def junk16():
            return junkp.tile([128, BLK], F16, tag="junk", name="junk")

        def tree_max(rg, out_ap):
            nn_ = BLK // DEG
            v = rg.rearrange("p (n w) -> p n w", n=nn_)
            t1 = ring.tile([128, nn_, 8], F16, tag="t1")
            nc.vector.tensor_max(t1, v[:, :, 0:8], v[:, :, 8:16])
            t2 = ring.tile([128, nn_, 4], F16, tag="t2")
            nc.vector.tensor_max(t2, t1[:, :, 0:4], t1[:, :, 4:8])
            t3 = ring.tile([128, nn_, 2], F16, tag="t3")
            nc.vector.tensor_max(t3, t2[:, :, 0:2], t2[:, :, 2:4])
            nc.vector.tensor_max(out_ap.rearrange("p (n w) -> p n w", n=nn_),
                                 t3[:, :, 0:1], t3[:, :, 1:2])

        # ============ conv1 ============
        with tc.tile_pool(name="p1", bufs=1) as P1, \
             tc.tile_pool(name="p1g", bufs=2) as P1G:
            e1s_t = P1.tile([128, NE // 16], I16)
            nc.sync.dma_start(out=e1s_t, in_=e1s_d[:, :])
            edst_t = P1.tile([128, NE // 16], I16)
            nc.sync.dma_start(out=edst_t, in_=edst_d[:, :])
            r11 = P1.tile([128, NE // 2], F16)
            r12 = P1.tile([128, NE // 2], F16)

            nblk11 = NE // (2 * BLK)        # 16
            sA11, qD11 = slots(1, nblk11, "s11")
            w1e = WC("w1ext")
            w1d = WC("w1dn")
            NBLK11E = NE // (2 * BLK)       # 16 blocks of 2048 edges
            for blk in range(NBLK11E):
                gx = P1G.tile([128, 1, 2 * BLK], F16, tag="gx", name="gx")
                gd = P1G.tile([128, 1, 2 * BLK], F16, tag="gd", name="gd")
                for gsub in range(2 * BLK // GCH):
                    col = (blk * 2 * BLK + gsub * GCH) // 16
                    nc.gpsimd.dma_gather(
                        out_ap=gx[:, :, gsub * GCH:(gsub + 1) * GCH],
                        in_ap=tablex[:, :],
                        idxs_ap=e1s_t[:, col:col + GCH // 16],
                        num_idxs=GCH, num_idxs_reg=GCH,
                        elem_size=128, transpose=True, queue_num=0)
                    nc.gpsimd.dma_gather(
                        out_ap=gd[:, :, gsub * GCH:(gsub + 1) * GCH],
                        in_ap=tablex[:, :],
                        idxs_ap=edst_t[:, col:col + GCH // 16],
                        num_idxs=GCH, num_idxs_reg=GCH,
                        elem_size=128, transpose=True, queue_num=0)
                ps = psA.tile([128, BLK], F32, tag="psA", name="ps")
                for q in range(2):
                    e0 = q * BLK
                    cps = ps[:, q * 512:(q + 1) * 512]
                    nc.tensor.matmul(cps[0:64, :], w1e, gx[:, 0, e0:e0 + 512],
                                     start=True, stop=False, tile_position=(0, 0))
                    nc.tensor.matmul(cps[0:64, :], w1d, gd[:, 0, e0:e0 + 512],
                                     start=False, stop=False, tile_position=(0, 0))
                    nc.tensor.matmul(cps[64:128, :], w1e,
                                     gx[:, 0, e0 + 512:e0 + 1024],
                                     start=True, stop=False, tile_position=(0, 64))
                    nc.tensor.matmul(cps[64:128, :], w1d,
                                     gd[:, 0, e0 + 512:e0 + 1024],
                                     start=False, stop=True, tile_position=(0, 64))
                rr = r11[:, blk * BLK:(blk + 1) * BLK]
                nc.scalar.activation(rr, ps, AF.Relu,
                                     accum_out=sA11[:, 0, blk:blk + 1])
                nc.vector.scalar_tensor_tensor(
                    out=junk16(), in0=rr, scalar=0.0, in1=rr,
                    op0=ALU.bypass, op1=ALU.mult,
                    accum_out=qD11[:, 0, blk:blk + 1])
            s11, t11 = stat_sync("bn11", sA11, qD11, WA("g11"), WA("be11"),
                                 NBAT_E, True, 1)
            w12f = fold_w(WA("w12d"), s11, 1, 64, "w12f")
            br12 = small.tile([1, 128], F16, tag="br12")
            pb12 = psS.tile([1, 512], F32, tag="ps_small")
            nc.tensor.matmul(pb12[:, 0:128], t11[0:64, 0:1], WA("w12s")[0:64, :],
                             start=True, stop=True)
            nc.vector.tensor_add(br12, pb12[:, 0:128], WA("b12r")[0:1, :])

            # L12
            sA12, qD12 = slots(1, nblk11, "s12")
            for blk in range(nblk11):
                ps = psA.tile([128, BLK], F32, tag="psA")
                for q in range(2):
                    cps = ps[:, q * 512:(q + 1) * 512]
                    c0 = blk * BLK + q * 512
                    rs = r11[:, c0:c0 + 512]
                    nc.tensor.matmul(cps[0:64, :], w12f[0:64, 0, :], rs[0:64, :],
                                     start=True, stop=False, tile_position=(0, 0))
                    nc.tensor.matmul(cps[64:128, :], w12f[64:128, 0, :],
                                     rs[64:128, :], start=True, stop=False,
                                     tile_position=(64, 64))
                    nc.tensor.matmul(cps, br12, ones16[:, 0:512],
                                     start=False, stop=True)
                rr = r12[:, blk * BLK:(blk + 1) * BLK]
                nc.scalar.activation(rr, ps, AF.Relu,
                                     accum_out=sA12[:, 0, blk:blk + 1])
                nc.vector.scalar_tensor_tensor(
                    out=junk16(), in0=rr, scalar=0.0, in1=rr,
                    op0=ALU.bypass, op1=ALU.mult, accum_out=qD12[:, 0, blk:blk + 1])
            s12, t12 = stat_sync("bn12", sA12, qD12, WA("g12"), WA("be12"),
                                 NBAT_E, True, 1)
            w13f = fold_w(WA("w13d"), s12, 1, 128, "w13f")
            br13 = bias_row(t12, WA("w13d"), WA("b13r"), 1, 128, 64, "br13")

            # L13: unpacked; aggregation
            nblk13 = NE // BLK              # 32
            sA13, qD13 = slots(1, nblk13, "s13")
            for blk in range(nblk13):
                ps = psA.tile([128, BLK], F32, tag="psA")
                for q in range(2):
                    g512 = 2 * blk + q
                    half = g512 % 2
                    col0 = (g512 // 4) * 1024 + ((g512 // 2) % 2) * 512
                    base = 64 * half
                    rs = r12[base:base + 64, col0:col0 + 512]
                    cps = ps[:, q * 512:(q + 1) * 512]
                    nc.tensor.matmul(cps, w13f[base:base + 64, 0, :], rs,
                                     start=True, stop=False, tile_position=(base, 0))
                    nc.tensor.matmul(cps, br13, ones16[:, 0:512],
                                     start=False, stop=True)
                rg = ring.tile([128, BLK], F16, tag="ring1")
                nc.scalar.activation(rg, ps, AF.Relu,
                                     accum_out=sA13[:, 0, blk:blk + 1])
                nc.vector.scalar_tensor_tensor(
                    out=junk16(), in0=rg, scalar=0.0, in1=rg,
                    op0=ALU.bypass, op1=ALU.mult, accum_out=qD13[:, 0, blk:blk + 1])
                nn_ = BLK // DEG
                tree_max(rg, agg1[:, blk * nn_:(blk + 1) * nn_])
            s13, t13 = stat_sync("bn13", sA13, qD13, WA("g13"), WA("be13"),
                                 NBAT_E, False, 1)

            # tables for conv2
            w2xf = fold_w(WA("w2x"), s13, 1, 128, "w2xf")
            brq2 = bias_row(t13, WA("w2x"), WA("b21r"), 1, 128, 128, "brq2")
            w2p4 = WC("w2p4")
            for ch in range(16):
                c0, c1 = ch * 128, (ch + 1) * 128
                ps = psS.tile([128, 128], F32, tag="ps_small")
                nc.tensor.matmul(ps, agg1[:, c0:c1], w2xf[:, 0, :],
                                 start=True, stop=False)
                nc.tensor.matmul(ps, pos4[:, c0:c1], w2p4, start=False, stop=False)
                nc.tensor.matmul(ps, ones16[:, c0:c1], brq2, start=False, stop=True)
                tq = small.tile([128, 128], F16, tag="tq")
                nc.scalar.copy(tq, ps)
                nc.sync.dma_start(out=bass.AP(
                    tensor=table2q[:, :].tensor, offset=ch * 128 * 128,
                    ap=[[128, 128], [1, 128]]), in_=tq)
            # rd2 (dst-side pos term) feature-major in SBUF: rd2f[f, v]
            for q in range(NN // 512):
                ps2 = psS.tile([128, 512], F32, tag="ps_small")
                nc.tensor.matmul(ps2, w2p4, pos4[:, q * 512:(q + 1) * 512],
                                 start=True, stop=True)
                nc.scalar.copy(rd2f[:, q * 512:(q + 1) * 512], ps2)

        # ============ conv2 ============
        nblk2 = NE // BLK                   # 32
        with tc.tile_pool(name="p2", bufs=1) as P2, \
             tc.tile_pool(name="p2g", bufs=2) as P2G, \
             tc.tile_pool(name="p2s", bufs=3) as P2S:
            e2s_t = P2.tile([128, NE // 16], I16)
            nc.sync.dma_start(out=e2s_t, in_=e2s_d[:, :])

            # L21: r21 = relu(q2[src] - rd2[dst]); spill to DRAM
            sA21, qD21 = slots(1, nblk2, "s21")
            for blk in range(nblk2):
                g2 = P2G.tile([128, 1, BLK], F16, tag="gx", name="g2")
                for gsub in range(BLK // GCH):
                    col = (blk * BLK + gsub * GCH) // 16
                    nc.gpsimd.dma_gather(
                        out_ap=g2[:, :, gsub * GCH:(gsub + 1) * GCH],
                        in_ap=table2q[:, :],
                        idxs_ap=e2s_t[:, col:col + GCH // 16],
                        num_idxs=GCH, num_idxs_reg=GCH,
                        elem_size=128, transpose=True, queue_num=0)
                rr = P2S.tile([128, BLK], F16, tag="rblk", name="rr")
                sub = P2S.tile([128, BLK], F16, tag="sub", name="sub")
                nnb = BLK // DEG
                rsl = rd2f[:, blk * nnb:(blk + 1) * nnb]
                rd_b = bass.AP(tensor=rsl.tensor, offset=rsl.offset,
                               ap=[rsl.ap[0], rsl.ap[1], [0, DEG]])
                nc.vector.tensor_sub(sub.rearrange("p (n w) -> p n w", n=nnb),
                                     g2[:, 0, :].rearrange("p (n w) -> p n w", n=nnb),
                                     rd_b)
                nc.scalar.activation(rr, sub, AF.Relu,
                                     accum_out=sA21[:, 0, blk:blk + 1])
                nc.vector.scalar_tensor_tensor(
                    out=junk16(), in0=rr, scalar=0.0, in1=rr,
                    op0=ALU.bypass, op1=ALU.mult,
                    accum_out=qD21[:, 0, blk:blk + 1])
                nc.sync.dma_start(out=bass.AP(
                    tensor=r21_dr[:, :].tensor, offset=blk * BLK,
                    ap=[[NE, 128], [1, BLK]]), in_=rr)
            s21, t21 = stat_sync("bn21", sA21, qD21, WA("g21"), WA("be21"),
                                 NBAT_E, False, 1)
            w22f = fold_w(WA("w22"), s21, 1, 128, "w22f")
            br22 = bias_row(t21, WA("w22"), WA("b22r"), 1, 128, 128, "br22")

            # L22: 128->128; stream r21 in, spill r22 out
            sA22, qD22 = slots(1, nblk2, "s22")
            for blk in range(nblk2):
                rin = P2S.tile([128, BLK], F16, tag="rin")
                nc.sync.dma_start(out=rin, in_=bass.AP(
                    tensor=r21_dr[:, :].tensor, offset=blk * BLK,
                    ap=[[NE, 128], [1, BLK]]))
                ps = psA.tile([128, BLK], F32, tag="psA")
                for q in range(2):
                    cps = ps[:, q * 512:(q + 1) * 512]
                    nc.tensor.matmul(cps, w22f[:, 0, :], rin[:, q * 512:(q + 1) * 512],
                                     start=True, stop=False)
                    nc.tensor.matmul(cps, br22, ones16[:, 0:512],
                                     start=False, stop=True)
                rr = P2S.tile([128, BLK], F16, tag="rblk")
                nc.scalar.activation(rr, ps, AF.Relu,
                                     accum_out=sA22[:, 0, blk:blk + 1])
                nc.vector.scalar_tensor_tensor(
                    out=junk16(), in0=rr, scalar=0.0, in1=rr,
                    op0=ALU.bypass, op1=ALU.mult, accum_out=qD22[:, 0, blk:blk + 1])
                nc.sync.dma_start(out=bass.AP(
                    tensor=r22_dr[:, :].tensor, offset=blk * BLK,
                    ap=[[NE, 128], [1, BLK]]), in_=rr)
            s22, t22 = stat_sync("bn22", sA22, qD22, WA("g22"), WA("be22"),
                                 NBAT_E, False, 1)
            w23f = fold_w(WA("w23"), s22, 1, 256, "w23f")
            br23 = bias_row(t22, WA("w23"), WA("b23r"), 1, 256, 128, "br23")

            # L23: 128->256; stream r22 in (per M-chunk); aggregate
            sA23, qD23 = slots(2, nblk2, "s23")
            for mc in range(2):
                for blk in range(nblk2):
                    rin = P2S.tile([128, BLK], F16, tag="rin")
                    nc.sync.dma_start(out=rin, in_=bass.AP(
                        tensor=r22_dr[:, :].tensor, offset=blk * BLK,
                        ap=[[NE, 128], [1, BLK]]))
                    ps = psA.tile([128, BLK], F32, tag="psA")
                    for q in range(2):
                        cps = ps[:, q * 512:(q + 1) * 512]
                        nc.tensor.matmul(cps, w23f[:, 0, mc * 128:(mc + 1) * 128],
                                         rin[:, q * 512:(q + 1) * 512],
                                         start=True, stop=False)
                        nc.tensor.matmul(cps, br23[:, mc * 128:(mc + 1) * 128],
                                         ones16[:, 0:512], start=False, stop=True)
                    rg = ring.tile([128, BLK], F16, tag="ring1")
                    nc.scalar.activation(rg, ps, AF.Relu,
                                         accum_out=sA23[:, mc, blk:blk + 1])
                    nc.vector.scalar_tensor_tensor(
                        out=junk16(), in0=rg, scalar=0.0, in1=rg,
                        op0=ALU.bypass, op1=ALU.mult,
                        accum_out=qD23[:, mc, blk:blk + 1])
                    nn_ = BLK // DEG
                    tree_max(rg, agg2[:, mc, blk * nn_:(blk + 1) * nn_])
            s23, t23 = stat_sync("bn23", sA23, qD23, WA("g23"), WA("be23"),
                                 NBAT_E, False, 2)

        # ============ mlp3 ============
        nblkN = NN // BLK                   # 2
        with tc.tile_pool(name="p3", bufs=1) as P3:
            b32b = P3.tile([128, CB], F32)
            nc.sync.dma_start(out=b32b, in_=b32b_d[:, :])

            def WB(name):
                c0, w, r = SB[name]
                return b32b[:r, c0:c0 + w]

            w3af = fold_w(WB("w3a"), s23, 2, 256, "w3af")
            br31 = bias_row(t23, WB("w3a"), WB("b31r"), 2, 256, 128, "br31")
            w3p4 = WC("w3p4")
            r31 = P3.tile([128, 2, NN], F16)
            sA31, qD31 = slots(2, nblkN, "s31")
            for mc in range(2):
                for blk in range(nblkN):
                    ps = psA.tile([128, BLK], F32, tag="psA")
                    for q in range(2):
                        cs = blk * BLK + q * 512
                        cps = ps[:, q * 512:(q + 1) * 512]
                        nc.tensor.matmul(cps, w3af[:, 0, mc * 128:(mc + 1) * 128],
                                         agg2[:, 0, cs:cs + 512], start=True, stop=False)
                        nc.tensor.matmul(cps, w3af[:, 1, mc * 128:(mc + 1) * 128],
                                         agg2[:, 1, cs:cs + 512], start=False, stop=False)
                        nc.tensor.matmul(cps, w3p4[:, mc * 128:(mc + 1) * 128],
                                         pos4[:, cs:cs + 512], start=False, stop=False)
                        nc.tensor.matmul(cps, br31[:, mc * 128:(mc + 1) * 128],
                                         ones16[:, 0:512], start=False, stop=True)
                    rr = r31[:, mc, blk * BLK:(blk + 1) * BLK]
                    nc.scalar.activation(rr, ps, AF.Relu,
                                         accum_out=sA31[:, mc, blk:blk + 1])
                    nc.vector.scalar_tensor_tensor(
                        out=junk16(), in0=rr, scalar=0.0, in1=rr,
                        op0=ALU.bypass, op1=ALU.mult,
                        accum_out=qD31[:, mc, blk:blk + 1])
            s31, t31 = stat_sync("bn31", sA31, qD31, WB("g31"), WB("be31"),
                                 NBAT_N, False, 2)

            w3bf = fold_w(WB("w3b"), s31, 2, 512, "w3bf")
            br32 = bias_row(t31, WB("w3b"), WB("b32r"), 2, 512, 128, "br32")
            r32 = P3.tile([128, 4, NN], F16)
            sA32, qD32 = slots(4, nblkN, "s32")
            for mc in range(4):
                for blk in range(nblkN):
                    ps = psA.tile([128, BLK], F32, tag="psA")
                    for q in range(2):
                        cs = blk * BLK + q * 512
                        cps = ps[:, q * 512:(q + 1) * 512]
                        for kc in range(2):
                            nc.tensor.matmul(cps, w3bf[:, kc, mc * 128:(mc + 1) * 128],
                                             r31[:, kc, cs:cs + 512],
                                             start=(kc == 0), stop=False)
                        nc.tensor.matmul(cps, br32[:, mc * 128:(mc + 1) * 128],
                                         ones16[:, 0:512], start=False, stop=True)
                    rr = r32[:, mc, blk * BLK:(blk + 1) * BLK]
                    nc.scalar.activation(rr, ps, AF.Relu,
                                         accum_out=sA32[:, mc, blk:blk + 1])
                    nc.vector.scalar_tensor_tensor(
                        out=junk16(), in0=rr, scalar=0.0, in1=rr,
                        op0=ALU.bypass, op1=ALU.mult,
                        accum_out=qD32[:, mc, blk:blk + 1])
            s32, t32 = stat_sync("bn32", sA32, qD32, WB("g32"), WB("be32"),
                                 NBAT_N, False, 4)

            w3cf = fold_w(WB("w3c"), s32, 4, 1024, "w3cf")
            br33 = bias_row(t32, WB("w3c"), WB("b33r"), 4, 1024, 128, "br33")
            xgp = statp.tile([128, 8, NG], F32, tag="xgp")
            sA33, qD33 = slots(8, nblkN, "s33")
            for mc in range(8):
                for blk in range(nblkN):
                    ps = psA.tile([128, BLK], F32, tag="psA")
                    for q in range(2):
                        cs = blk * BLK + q * 512
                        cps = ps[:, q * 512:(q + 1) * 512]
                        for kc in range(4):
                            nc.tensor.matmul(cps, w3cf[:, kc, mc * 128:(mc + 1) * 128],
                                             r32[:, kc, cs:cs + 512],
                                             start=(kc == 0), stop=False)
                        nc.tensor.matmul(cps, br33[:, mc * 128:(mc + 1) * 128],
                                         ones16[:, 0:512], start=False, stop=True)
                    rg = ring.tile([128, BLK], F16, tag="ring1")
                    nc.scalar.activation(rg, ps, AF.Relu,
                                         accum_out=sA33[:, mc, blk:blk + 1])
                    nc.vector.scalar_tensor_tensor(
                        out=junk16(), in0=rg, scalar=0.0, in1=rg,
                        op0=ALU.bypass, op1=ALU.mult,
                        accum_out=qD33[:, mc, blk:blk + 1])
                    ngb = BLK // PTS
                    nc.vector.reduce_max(
                        xgp[:, mc, blk * ngb:(blk + 1) * ngb],
                        rg.rearrange("p (g n) -> p g n", g=ngb), axis=AX.X)

            # batched bn33 stats + pooled features AllGather
            stf = statp.tile([128, 80], F32, tag="stf")
            for c in range(8):
                nc.vector.reduce_sum(stf[:, 2 * c:2 * c + 1], sA33[:, c, :], axis=AX.X)
                nc.vector.reduce_sum(stf[:, 2 * c + 1:2 * c + 2], qD33[:, c, :],
                                     axis=AX.X)
            nc.vector.tensor_copy(stf[:, 16:80], xgp.rearrange("p a b -> p (a b)"))
            nc.sync.dma_start(out=cc_in["bn33x"][:, :], in_=stf)
            nc.gpsimd.collective_compute(
                "AllGather", ALU.bypass, ins=[cc_in["bn33x"][:, :]],
                outs=[cc_out["bn33x"][:, :, :]], replica_groups=RG)
            w = 80
            allst = statp.tile([128, NCORE, 16], F32, tag="all33")
            nc.sync.dma_start(out=allst, in_=bass.AP(
                tensor=cc_out["bn33x"][:, :, :].tensor, offset=0,
                ap=[[w, 128], [128 * w, NCORE], [1, 16]]))
            tot33 = statp.tile([128, 16], F32, tag="tot33")
            aswap33 = bass.AP(tensor=allst.tensor, offset=allst.offset,
                              ap=[allst.ap[0], [1, 16], [16, NCORE]])
            nc.vector.reduce_sum(tot33, aswap33, axis=AX.X)
            nc.vector.tensor_scalar_mul(tot33, tot33, 1.0 / NBAT_N)
            s33 = statp.tile([128, 8], F32, tag="s33")
            t33 = statp.tile([128, 8], F32, tag="t33")
            for c in range(8):
                mn = tot33[:, 2 * c:2 * c + 1]
                ex2 = tot33[:, 2 * c + 1:2 * c + 2]
                var = small.tile([128, 1], F32, tag="var")
                nc.vector.tensor_mul(var, mn, mn)
                nc.vector.tensor_sub(var, ex2, var)
                lnv = small.tile([128, 1], F32, tag="lnv")
                nc.scalar.activation(lnv, var, AF.Ln, bias=eps_t)
                rstd = small.tile([128, 1], F32, tag="rstd")
                nc.scalar.activation(rstd, lnv, AF.Exp, scale=-0.5)
                nc.vector.tensor_mul(s33[:, c:c + 1], WB("g33")[:, c:c + 1], rstd)
                ms = small.tile([128, 1], F32, tag="ms")
                nc.vector.tensor_mul(ms, mn, s33[:, c:c + 1])
                nc.vector.tensor_sub(t33[:, c:c + 1], WB("be33")[:, c:c + 1], ms)
            t33h = statp.tile([128, 8], F16, tag="t33h")
            nc.vector.tensor_copy(t33h, t33)

            # ============ head (replicated) ============
            b16b = P3.tile([128, CD], F16)
            nc.sync.dma_start(out=b16b, in_=b16b_d[:, :])

            def WD(name):
                c0, w_, r = SD[name]
                return b16b[:r, c0:c0 + w_]

            xga = statp.tile([128, NCORE, 64], F32, tag="xgar")
            nc.sync.dma_start(out=xga, in_=bass.AP(
                tensor=cc_out["bn33x"][:, :, :].tensor, offset=16,
                ap=[[w, 128], [128 * w, NCORE], [1, 64]]))
            xg = statp.tile([128, 8, B], F16, tag="xg")
            for kc in range(8):
                nc.scalar.activation(
                    xg[:, kc, :].rearrange("p (a b) -> p a b", a=NCORE),
                    xga[:, :, kc * NG:(kc + 1) * NG],
                    AF.Relu, scale=s33[:, kc:kc + 1])

            lin1h = WD("lin1")
            lin2h = WD("lin2")
            regh = WD("reg")
            o1 = statp.tile([128, 4, B], F16, tag="o1")
            for m in range(4):
                # t33 contribution to pre-BN mean shift: z = xg@W + t33@W (+b);
                # BN removes constant shifts, so only s33-scaled xg matters for
                # the centered value -- but t33@W shifts the mean, variance
                # unchanged; bias b absorbed by BN. Compute column shift:
                ps = psS.tile([128, B], F32, tag="ps_small")
                for kc in range(8):
                    nc.tensor.matmul(ps, lin1h[:, kc * 512 + m * 128:kc * 512 + (m + 1) * 128],
                                     xg[:, kc, :], start=(kc == 0), stop=(kc == 7))
                bs = small.tile([128, 6], F32, tag="bs")
                nc.vector.bn_stats(bs, ps)
                mv = small.tile([128, 2], F32, tag="mv")
                nc.vector.bn_aggr(mv, bs)
                lnv = small.tile([128, 1], F32, tag="lnv")
                nc.scalar.activation(lnv, mv[:, 1:2], AF.Ln, bias=eps_t)
                rstd = small.tile([128, 1], F32, tag="rstd")
                nc.scalar.activation(rstd, lnv, AF.Exp, scale=-0.5)
                sh = small.tile([128, 1], F32, tag="sh")
                nc.vector.tensor_mul(sh, WB("gh1")[:, m:m + 1], rstd)
                th = small.tile([128, 1], F32, tag="th")
                nc.vector.tensor_mul(th, mv[:, 0:1], sh)
                nc.vector.tensor_sub(th, WB("bh1")[:, m:m + 1], th)
                nc.scalar.activation(o1[:, m, :], ps, AF.Relu, scale=sh, bias=th)

            o2 = statp.tile([128, 4, B], F16, tag="o2")
            for m in range(4):
                ps = psS.tile([128, B], F32, tag="ps_small")
                for kc in range(4):
                    nc.tensor.matmul(ps, lin2h[:, kc * 512 + m * 128:kc * 512 + (m + 1) * 128],
                                     o1[:, kc, :], start=(kc == 0), stop=(kc == 3))
                bs = small.tile([128, 6], F32, tag="bs")
                nc.vector.bn_stats(bs, ps)
                mv = small.tile([128, 2], F32, tag="mv")
                nc.vector.bn_aggr(mv, bs)
                lnv = small.tile([128, 1], F32, tag="lnv")
                nc.scalar.activation(lnv, mv[:, 1:2], AF.Ln, bias=eps_t)
                rstd = small.tile([128, 1], F32, tag="rstd")
                nc.scalar.activation(rstd, lnv, AF.Exp, scale=-0.5)
                sh = small.tile([128, 1], F32, tag="sh")
                nc.vector.tensor_mul(sh, WB("gh2")[:, m:m + 1], rstd)
                th = small.tile([128, 1], F32, tag="th")
                nc.vector.tensor_mul(th, mv[:, 0:1], sh)
                nc.vector.tensor_sub(th, WB("bh2")[:, m:m + 1], th)
                nc.scalar.activation(o2[:, m, :], ps, AF.Relu, scale=sh, bias=th)

            pso = psS.tile([1, B], F32, tag="ps_small")
            for kc in range(4):
                nc.tensor.matmul(pso, regh[:, kc:kc + 1], o2[:, kc, :],
                                 start=(kc == 0), stop=(kc == 3))
            outt = small.tile([1, B], F32, tag="outt")
            nc.scalar.activation(outt, pso, AF.Identity, bias=WB("regb")[0:1, 0:1])
            nc.sync.dma_start(out=bass.AP(tensor=out_d[:, :].tensor, offset=0,
                                          ap=[[0, 1], [1, B]]), in_=outt)

    nc.compile()
    return nc


_CACHE = {}


def kernel(**inputs):
    in_maps, shapes = _host_prep(inputs)
    if "nc" not in _CACHE:
        nc = bacc.Bacc()
        build_kernel(nc, shapes)
        _CACHE["nc"] = nc
    res = run_bass_kernel_spmd(nc := _CACHE["nc"], in_maps, list(range(NCORE)))
    return np.ascontiguousarray(np.asarray(res.results[0]["out"], np.float32))
